# revision 18
# baseline (speedup 1.0000x reference)
"""CrossModalAttention fused Bass/Tile kernel for Trainium2 (8 NeuronCores).

Math (per batch b):
    pooled = mean_w x_skel[b]                      # [Cs, Ws]
    k  = Wk @ pooled + bk                          # [Ci, Ws]
    q  = Wq @ x_rgb[b] + bq                        # (never materialized)
    energy = q^T k = x_rgb^T (Wq^T k) + 1 (bq^T k) # [HW, Ws]  <- low-rank trick
    att = softmax(energy, axis=-1)
    v  = Wv @ pooled + bv
    out = gamma * (v @ att^T) + x_rgb

Weight-only host folds (exact algebra, fp64 accumulation):
    Wkq = (Wk/25)^T Wq   [Cs, Cr]   so  Wq^T k = Wkq^T pooled_sum + bkq
    bkq = Wq^T bk        [Cr]
    u   = Wk^T bq / 25   [Cs]       so  bq^T k = u^T pooled_sum + cbb
    cbb = bq . bk        scalar
    WvT = gamma (Wv/25)^T [Cs, Cr]  (gamma folded into v)
    gbv = gamma bv       [Cr]       (row 25 of the vT stationary)

Structure (v3):
  * Everything heavy is bf16: HBM traffic is 8.1MB/core (vs 16.1 fp32)
    and every PE matmul streams single-pass. Host pre-packs all tensors
    into exact SBUF layouts so DMAs are 128 contiguous descriptors.
  * 8 dummy warm-up matmuls run while inputs stream in, so the PE HAM
    clock-gate is at 2.4GHz before the first real matmul.
  * kq is built channel-on-partition directly (contraction over Cs);
    bq^T k rides the exp() ACT as a per-partition bias.
  * softmax: s = ones^T E (PE), r = 1/s (DVE), R = bcast r (PE),
    attT = E*R (DVE). attT row 25 = ones and vT row 25 = gamma*bv, so
    the out matmul adds the bias for free (K=26 costs the same as 25).
  * residual adds (PSUM + x -> bf16 out) are split per channel-block
    between DVE (tensor_add) and an ACT-copy + GpSimd-add crew; PSUM
    copies of kq/vT/be ride DVE early, keeping ACT for the exps.
  * PE queue is emission-ordered so energy(b1) h0 fills the gap between
    softmax(b0) and out(b0) instead of idling on DMA.

Sharding: pure data-parallel over batch B=16 -> 2 batches per NeuronCore.
"""

import os
import sys

for _p in ("/opt/trn_rl_repo", "/root/.axon_site/_ro/trn_rl_repo"):
    if os.path.isdir(_p) and _p not in sys.path:
        sys.path.insert(0, _p)

import ml_dtypes
import numpy as np

import concourse.bass as bass  # noqa: F401
import concourse.mybir as mybir
import concourse.tile as tile
from concourse import bacc
from concourse.bass_utils import run_bass_kernel_spmd
from concourse.masks import make_identity

B, Cr, H, W = 16, 1024, 28, 28
Cs, Hs, Ws = 256, 25, 25
Ci = 512
HW = H * W  # 784
SK = Hs * Ws  # 625
N_CORES = 8
BPC = B // N_CORES  # batches per core = 2
WA = Ws + 1  # 26 rows: 0..24 att, 25 = ones/bias row
NT = (512, 272)  # free-dim tiling of HW=784, PSUM-bank aligned
DVE_RTS = (0, 2, 4, 6)  # channel blocks whose residual add rides DVE
N_WARM = 20  # dummy PE matmuls: flip + hold the HAM clock gate at 2.4GHz
FP = mybir.dt.float32
BF = mybir.dt.bfloat16
AX = mybir.AxisListType
AF = mybir.ActivationFunctionType
ALU = mybir.AluOpType
BF_NP = ml_dtypes.bfloat16


def _nt_slices():
    off = 0
    for n in NT:
        yield off, n
        off += n


def _build():
    nc = bacc.Bacc(None, target_bir_lowering=False)

    xr_d = nc.dram_tensor("xr", [128, BPC * 8 * HW], BF, kind="ExternalInput")
    xs_d = nc.dram_tensor("xs", [128, 2 * BPC * SK], BF, kind="ExternalInput")
    Wkq_d = nc.dram_tensor("Wkq", [128, 2 * Cr], BF, kind="ExternalInput")
    WvT_d = nc.dram_tensor("WvT", [128, 2 * Cr], BF, kind="ExternalInput")
    bkq_d = nc.dram_tensor("bkq", [1, Cr], BF, kind="ExternalInput")
    # smb cols: 0..1 = u (ct halves), 2 = cbb (replicated), 3 = pad
    smb_d = nc.dram_tensor("smb", [128, 4], BF, kind="ExternalInput")
    gbv_d = nc.dram_tensor("gbv", [1, Cr], BF, kind="ExternalInput")
    out_d = nc.dram_tensor("out", [128, BPC * 8 * HW], BF, kind="ExternalOutput")

    xr_dv = xr_d.rearrange("p (b t n) -> p b t n", b=BPC, t=8)
    xs_dv = xs_d.rearrange("p (c b j) -> p c b j", c=2, b=BPC)
    out_dv = out_d.rearrange("p (b t n) -> p b t n", b=BPC, t=8)

    with tile.TileContext(nc) as tc:
        with (
            nc.allow_low_precision(reason="bf16 pipeline (tolerance 2e-2)"),
            tc.tile_pool(name="const", bufs=1) as const,
            tc.tile_pool(name="wt", bufs=1) as wt,
            tc.tile_pool(name="xp", bufs=2) as xp,
            tc.tile_pool(name="work", bufs=2) as work,
            tc.tile_pool(name="outp", bufs=2) as outp,
            tc.tile_pool(name="psE", bufs=3, space="PSUM") as psE,
            tc.tile_pool(name="psS", bufs=2, space="PSUM") as psS,
            tc.tile_pool(name="psO", bufs=3, space="PSUM") as psO,
        ):
            # ---- consts first on gpsimd (gate the PE warm-up matmuls)
            warm_src = const.tile([1, 512], BF, tag="warm_src")
            nc.gpsimd.memset(warm_src, 1.0)
            ones_r = const.tile([1, 64], BF, tag="ones_r")
            nc.gpsimd.memset(ones_r, 1.0)
            ones_c = const.tile([Ws, 1], BF, tag="ones_c")
            nc.gpsimd.memset(ones_c, 1.0)
            ident = const.tile([128, 128], BF, tag="ident")
            make_identity(nc, ident)

            # ---- input DMAs. sync carries x_skel then all x_rgb; weights
            # ride the gpsimd/scalar-issued queues in parallel.
            xs_sb = wt.tile([128, 2, BPC, SK], BF, tag="xs")
            nc.sync.dma_start(xs_sb, xs_dv)
            x_sbs = []  # [b][half] -> [128, 4, HW]
            for b in range(BPC):
                halves = []
                for h in range(2):
                    x_sb = xp.tile([128, 4, HW], BF, tag=f"x{h}", name=f"x{b}_{h}")
                    nc.sync.dma_start(x_sb, xr_dv[:, b, h * 4 : (h + 1) * 4, :])
                    halves.append(x_sb)
                x_sbs.append(halves)
            Wkq_sb = wt.tile([128, 2, Cr], BF, tag="wkq")
            nc.gpsimd.dma_start(Wkq_sb, Wkq_d.rearrange("p (c r) -> p c r", c=2))
            bkq_row = wt.tile([1, Cr], BF, tag="bkq")
            nc.gpsimd.dma_start(bkq_row, bkq_d[:])
            WvT_sb = wt.tile([128, 2, Cr], BF, tag="wvt")
            nc.scalar.dma_start(WvT_sb, WvT_d.rearrange("p (c r) -> p c r", c=2))
            smb_sb = wt.tile([128, 4], BF, tag="smb")
            nc.scalar.dma_start(smb_sb, smb_d[:])

            # vT stationaries [WA, Cr]: row 25 = gamma*bv lands via DMA,
            # rows 0..24 come from the v matmul PSUM copies below.
            vT_sbs = []
            for b in range(BPC):
                vT = wt.tile([WA, Cr], BF, tag=f"vT{b}", name=f"vT{b}")
                nc.sync.dma_start(vT[25:26, :], gbv_d[:])
                vT_sbs.append(vT)

            # per-batch softmax work tiles; attT row 25 = exact ones
            E_sbs, attTs, r32s, rbfs = [], [], [], []
            for b in range(BPC):
                E_sb = work.tile([Ws, HW], BF, tag="E", name=f"E{b}")
                attT = work.tile([WA, HW], BF, tag="attT", name=f"attT{b}")
                # row 25 must be exact ones (bias row of the out matmul);
                # partition-25 slices aren't addressable by compute engines,
                # so memset the whole tile — rows 0..24 get overwritten.
                nc.vector.memset(attT, 1.0)
                r32 = work.tile([1, HW], FP, tag="r32", name=f"r32_{b}")
                rbf = work.tile([1, HW], BF, tag="rbf", name=f"rbf{b}")
                E_sbs.append(E_sb)
                attTs.append(attT)
                r32s.append(r32)
                rbfs.append(rbf)

            # ---- PE warm-up: ~3.4us of matmuls flips the HAM clock gate
            # (1.2 -> 2.4 GHz); the rest hold it warm until kq is ready.
            for i in range(N_WARM):
                wp = psS.tile([1, 512], FP, tag="ps", name=f"warm{i}")
                nc.tensor.matmul(
                    wp, ones_r[0:1, 0:1], warm_src, start=True, stop=True
                )

            # ---- pooled_sum [128(cs%), ct, b, Ws] (1/25 folded in weights)
            pooled = wt.tile([128, 2, BPC, Ws], BF, tag="pooled")
            nc.vector.reduce_sum(
                pooled,
                xs_sb.rearrange("p c b (h w) -> p c b h w", w=Ws),
                axis=AX.X,
            )

            # ---- kq [128(c%), kt, b*Ws+h] = Wkq^T pooled + bkq
            kq_sb = wt.tile([128, 8, BPC * Ws], BF, tag="kq")
            for kt in range(8):
                ps = psS.tile([128, BPC * Ws], FP, tag="ps", name=f"kq{kt}")
                nc.tensor.matmul(
                    ps,
                    bkq_row[0:1, kt * 128 : (kt + 1) * 128],
                    ones_r[0:1, 0 : BPC * Ws],
                    start=True,
                    stop=False,
                )
                for ct in range(2):
                    nc.tensor.matmul(
                        ps,
                        Wkq_sb[:, ct, kt * 128 : (kt + 1) * 128],
                        pooled[:, ct],
                        start=False,
                        stop=(ct == 1),
                    )
                nc.scalar.copy(kq_sb[:, kt], ps)

            # ---- be[s, b] = u^T pooled + cbb  (exp's per-partition bias)
            be_sb = wt.tile([Ws, BPC], FP, tag="be")
            for b in range(BPC):
                ps = psS.tile([Ws, 1], FP, tag="ps", name=f"be{b}")
                nc.tensor.matmul(
                    ps, ones_r[0:1, 0:Ws], smb_sb[0:1, 2:3], start=True, stop=False
                )
                for ct in range(2):
                    nc.tensor.matmul(
                        ps,
                        pooled[:, ct, b],
                        smb_sb[:, ct : ct + 1],
                        start=False,
                        stop=(ct == 1),
                    )
                nc.scalar.copy(be_sb[:, b : b + 1], ps)

            # ---- vT rows 0..24 = gamma v^T (gamma folded in WvT)
            for b in range(BPC):
                for nof in (0, 512):
                    ps = psS.tile([Ws, 512], FP, tag="ps", name=f"v{b}_{nof}")
                    for ct in range(2):
                        nc.tensor.matmul(
                            ps,
                            pooled[:, ct, b],
                            WvT_sb[:, ct, nof : nof + 512],
                            start=(ct == 0),
                            stop=(ct == 1),
                        )
                    nc.scalar.copy(vT_sbs[b][0:Ws, nof : nof + 512], ps)

            # ---- helpers ----------------------------------------------
            def energy_mms(b, eTs, kts):
                for kt in kts:
                    for (nof, nn), eT in zip(_nt_slices(), eTs):
                        nc.tensor.matmul(
                            eT,
                            kq_sb[:, kt, b * Ws : (b + 1) * Ws],
                            x_sbs[b][kt // 4][:, kt % 4, nof : nof + nn],
                            start=(kt == 0),
                            stop=(kt == 7),
                        )

            def softmax(b, eTs):
                for (nof, nn), eT in zip(_nt_slices(), eTs):
                    nc.scalar.activation(
                        E_sbs[b][:, nof : nof + nn],
                        eT,
                        func=AF.Exp,
                        bias=be_sb[:, b : b + 1],
                        scale=1.0,
                    )
                    srow = psS.tile([1, nn], FP, tag="ps", name=f"s{b}_{nof}")
                    nc.tensor.matmul(
                        srow,
                        ones_c,
                        E_sbs[b][:, nof : nof + nn],
                        start=True,
                        stop=True,
                    )
                    nc.vector.reciprocal_approx_fast(
                        r32s[b][:, nof : nof + nn], srow
                    )
                    nc.scalar.copy(
                        rbfs[b][:, nof : nof + nn], r32s[b][:, nof : nof + nn]
                    )
                    Rps = psS.tile([Ws, nn], FP, tag="ps", name=f"R{b}_{nof}")
                    nc.tensor.matmul(
                        Rps,
                        ones_r[0:1, 0:Ws],
                        rbfs[b][0:1, nof : nof + nn],
                        start=True,
                        stop=True,
                    )
                    nc.vector.tensor_mul(
                        attTs[b][0:Ws, nof : nof + nn],
                        E_sbs[b][:, nof : nof + nn],
                        Rps,
                    )

            def out_phase(b, o_sbs):
                for rt in range(8):
                    for nof, nn in _nt_slices():
                        ps = psO.tile(
                            [128, nn], FP, tag="op", name=f"op{b}_{rt}_{nof}"
                        )
                        dve = rt in DVE_RTS
                        xin = x_sbs[b][rt // 4][:, rt % 4, nof : nof + nn]
                        nc.tensor.matmul(
                            ps,
                            vT_sbs[b][:, rt * 128 : (rt + 1) * 128],
                            attTs[b][:, nof : nof + nn],
                            start=True,
                            stop=dve,
                        )
                        if not dve:
                            # accumulate the residual on the PE (identity
                            # stationary streams x); ACT then just copies
                            nc.tensor.matmul(
                                ps, ident, xin, start=False, stop=True
                            )
                        dst = o_sbs[rt // 4][:, rt % 4, nof : nof + nn]
                        if dve:
                            nc.vector.tensor_add(dst, ps, xin)
                        else:
                            nc.scalar.copy(dst, ps)

            # ---- main pipeline ----------------------------------------
            eTs0 = [
                psE.tile([Ws, nn], FP, tag="eT", name=f"eT0_{i}")
                for i, (_, nn) in enumerate(_nt_slices())
            ]
            eTs1 = [
                psE.tile([Ws, nn], FP, tag="eT", name=f"eT1_{i}")
                for i, (_, nn) in enumerate(_nt_slices())
            ]
            o0 = [
                outp.tile([128, 4, HW], BF, tag=f"o{h}", name=f"o0_{h}")
                for h in range(2)
            ]
            o1 = [
                outp.tile([128, 4, HW], BF, tag=f"o{h}", name=f"o1_{h}")
                for h in range(2)
            ]

            energy_mms(0, eTs0, range(8))
            softmax(0, eTs0)
            energy_mms(1, eTs1, range(4))  # fills PE while softmax(0) drains
            out_phase(0, o0)
            for h in range(2):
                nc.sync.dma_start(out_dv[:, 0, h * 4 : (h + 1) * 4, :], o0[h])
            energy_mms(1, eTs1, range(4, 8))
            softmax(1, eTs1)
            out_phase(1, o1)
            for h in range(2):
                nc.sync.dma_start(out_dv[:, 1, h * 4 : (h + 1) * 4, :], o1[h])

    nc.compile()
    return nc


_NC = None


def _get_nc():
    global _NC
    if _NC is None:
        _NC = _build()
    return _NC


def prepare_in_maps(x_rgb, x_skel, Wq, bq, Wk, bk, Wv, bv, gamma):
    """Host-side weight folds (weights only, exact fp64 algebra), dtype
    demotion to bf16, and SBUF-layout packing + per-core slicing."""
    xr = (
        np.asarray(x_rgb, np.float32)
        .reshape(B, 8, 128, HW)
        .transpose(2, 0, 1, 3)
        .astype(BF_NP)
    )  # [128(p), B, 8(t), HW]; channel c = t*128 + p
    xs = (
        np.asarray(x_skel, np.float32)
        .reshape(B, 2, 128, SK)
        .transpose(2, 1, 0, 3)
        .astype(BF_NP)
    )  # [128(p), 2(ct), B, SK]; cs = ct*128 + p
    Wq64 = np.asarray(Wq, np.float64)
    Wk64 = np.asarray(Wk, np.float64)
    Wv64 = np.asarray(Wv, np.float64)
    bq64 = np.asarray(bq, np.float64)
    bk64 = np.asarray(bk, np.float64)
    g64 = float(np.asarray(gamma, np.float64)[0])
    Wkq = (Wk64 / Ws).T @ Wq64  # [Cs, Cr]
    WvT = g64 * (Wv64 / Ws).T  # [Cs, Cr]
    bkq = Wq64.T @ bk64  # [Cr]
    u = Wk64.T @ bq64 / Ws  # [Cs]
    cbb = float(bq64 @ bk64)
    gbv = g64 * np.asarray(bv, np.float64)  # [Cr]

    Wkq_p = (
        np.ascontiguousarray(Wkq.reshape(2, 128, Cr).transpose(1, 0, 2))
        .reshape(128, 2 * Cr)
        .astype(BF_NP)
    )
    WvT_p = (
        np.ascontiguousarray(WvT.reshape(2, 128, Cr).transpose(1, 0, 2))
        .reshape(128, 2 * Cr)
        .astype(BF_NP)
    )
    bkq_p = bkq.reshape(1, Cr).astype(BF_NP)
    smb = np.zeros((128, 4), np.float64)
    smb[:, 0:2] = u.reshape(2, 128).T
    smb[:, 2] = cbb
    smb_p = smb.astype(BF_NP)
    gbv_p = gbv.reshape(1, Cr).astype(BF_NP)

    shared = {
        "Wkq": Wkq_p,
        "WvT": WvT_p,
        "bkq": bkq_p,
        "smb": smb_p,
        "gbv": gbv_p,
    }
    return [
        {
            "xr": np.ascontiguousarray(xr[:, c * BPC : (c + 1) * BPC]).reshape(
                128, BPC * 8 * HW
            ),
            "xs": np.ascontiguousarray(
                xs[:, :, c * BPC : (c + 1) * BPC]
            ).reshape(128, 2 * BPC * SK),
            **shared,
        }
        for c in range(N_CORES)
    ]


def kernel(x_rgb, x_skel, Wq, bq, Wk, bk, Wv, bv, gamma):
    nc = _get_nc()
    in_maps = prepare_in_maps(x_rgb, x_skel, Wq, bq, Wk, bk, Wv, bv, gamma)
    res = run_bass_kernel_spmd(nc, in_maps, core_ids=list(range(N_CORES)))
    outs = [
        np.asarray(r["out"])
        .reshape(128, BPC, 8, HW)
        .astype(np.float32)
        .transpose(1, 2, 0, 3)
        .reshape(BPC, Cr, H, W)
        for r in res.results
    ]
    return np.concatenate(outs, axis=0)
